# revision 21
# baseline (speedup 1.0000x reference)
"""MoE (top-2 of 8 experts, D=768, FF=3072) on 8 Trainium2 NeuronCores.

Strategy: FF-slice parallelism. The router runs on host; every core holds a
384-wide slice of D_FF for ALL 8 experts (same 9.4MB of fp16 weights per core
as expert-parallel) and runs the FFN for ALL routed token-expert pairs over
its slice, one expert "phase" at a time. Each core therefore does exactly
sum(C_e)*36 matmul-cycles -- perfect load balance with zero padding (vs
8*max(C_e)*36 for expert-parallel). The host sums the 8 partial outputs
(fp16) and applies the softmax-weighted combine + b2.

Device layout puts tokens on the matmul free axis, so both matmuls contract
over the partition axis with zero on-device transposes:
    HT[f,t] = relu(sum_d W1[d,f] * XT[d,t] + b1[f])   lhsT=W1, rhs=XT
    YT[d,t] =      sum_{f in slice} W2[f,d] * HT[f,t] lhsT=W2, rhs=HT
The chunk loop is software-pipelined (MM1 of chunk i+1 is emitted before
MM2 of chunk i) so the relu latency never stalls the PE. MM2's PSUM
accumulation is only 3 deep here, so the drain load is 8x expert-parallel's;
relu runs on ACT and the MM2 drains split 4:2 between DVE and ACT (Pool
cannot read PSUM). Tokens stream on the sync DMA queue, weights on the DVE
queue (parallel transfer in the prologue), outputs issue from Pool. The
last items' outputs go out per-2ko so the final transfers overlap the tail.
"""

import numpy as np

import concourse.tile as tile
from concourse import bacc, mybir
from concourse import bass_utils

D_MODEL = 768
N_EXPERTS = 8
TOP_K = 2
D_FF = 3072
P = 128
KO = D_MODEL // P     # 6   contraction chunks for MM1 / output tiles for MM2
FS = 3                # f-tiles per core slice (384 of 3072 FF columns)
FSP = FS * P          # 384
N0 = 256              # first-phase chunk0 (small so its tokens land early)
WARMUP_MMS = 40       # small 128-row dummy matmuls bridge the DMA prologue
TAIL_SPLIT = 3        # last N work items DMA their output per-2ko

_program_cache: dict[tuple, object] = {}


def _phase_chunks(C, first=False, last=False):
    """Chunks (t0, nt) covering C tokens, each <=512. The first phase opens
    with an N0 chunk (the first token DMA); the last phase switches to
    256-token chunks after the first so its output transfers drain early."""
    chunks = []
    t = 0
    if first and C > N0:
        chunks.append((0, N0))
        t = N0
    while t < C:
        n = min(512, C - t)
        chunks.append((t, n))
        t += n
    return chunks


def _build_program(counts):
    """Bass program: all 8 expert phases over this core's FF slice (SPMD x8).

    counts: per-phase token counts (already padded to x4)."""
    key = tuple(counts)
    if key in _program_cache:
        return _program_cache[key]

    fp16 = mybir.dt.float16
    fp32 = mybir.dt.float32
    nc = bacc.Bacc("TRN2", target_bir_lowering=False, debug=False,
                   enable_asserts=False, num_devices=N_EXPERTS)

    cmax = max(counts)
    c0 = counts[0]
    # wb_e layout: w1_e (ko-major [KO, FSP]) then w2_e ([FS, D])
    off_w2 = KO * FSP
    lw = off_w2 + FS * D_MODEL

    # phase-0 weights split into three need-ordered pieces so the first
    # matmul group is gated by the smallest possible transfer: w0a = w1_0
    # f-tile 0 + all biases (fp16), w0b = w1_0 f-tiles 1,2, w0c = w2_0.
    # All tokens ride the sync queue in FIFO (= need) order; all weights
    # ride the scalar queue. A queue's FIFO is what actually paces
    # transfers -- engines race ahead, so emission position alone is no
    # throttle.
    off_b1 = KO * P
    l0a = off_b1 + N_EXPERTS * FS
    xta_d = nc.dram_tensor("xta", [P, KO, N0], fp16,
                           kind="ExternalInput").ap()
    xtb_d = nc.dram_tensor("xtb", [P, KO, c0 - N0], fp16,
                           kind="ExternalInput").ap()
    w0a_d = nc.dram_tensor("w0a", [P, l0a], fp16,
                           kind="ExternalInput").ap()
    w0b_d = nc.dram_tensor("w0b", [P, KO, (FS - 1) * P], fp16,
                           kind="ExternalInput").ap()
    w0c_d = nc.dram_tensor("w0c", [P, FS, D_MODEL], fp16,
                           kind="ExternalInput").ap()
    wb_d = [None] + [
        nc.dram_tensor(f"wb{e}", [P, lw], fp16, kind="ExternalInput").ap()
        for e in range(1, N_EXPERTS)]
    xt_d = [None] + [
        nc.dram_tensor(f"xt{e}", [P, KO, counts[e]], fp16,
                       kind="ExternalInput").ap()
        for e in range(1, N_EXPERTS)]
    yt_d = [nc.dram_tensor(f"yt{e}", [P, KO, counts[e]], fp16,
                           kind="ExternalOutput").ap()
            for e in range(N_EXPERTS)]

    with tile.TileContext(nc) as tc:
        with (
            tc.tile_pool(name="wpool", bufs=1) as wpool,
            tc.tile_pool(name="xtpool", bufs=4) as xtpool,
            tc.tile_pool(name="hpool", bufs=2) as hpool,
            tc.tile_pool(name="ypool", bufs=3) as ypool,
            tc.tile_pool(name="pspool", bufs=7, space="PSUM") as pspool,
        ):
            xta_sb = wpool.tile([P, KO, N0], fp16)
            xtb_sb = wpool.tile([P, KO, c0 - N0], fp16)
            w0a_sb = wpool.tile([P, l0a], fp16)
            w0b_sb = wpool.tile([P, KO, (FS - 1) * P], fp16)
            w0c_sb = wpool.tile([P, FS, D_MODEL], fp16)
            wb_sb = [None] + [wpool.tile([P, lw], fp16, name=f"wb_sb{e}")
                              for e in range(1, N_EXPERTS)]

            def w1_slice(e, ft, ko):
                """AP of w1 f-tile `ft` (local), contraction chunk ko."""
                if e == 0:
                    if ft == 0:
                        return w0a_sb[:, ko * P:(ko + 1) * P]
                    return w0b_sb[:, ko, (ft - 1) * P:ft * P]
                o = ko * FSP + ft * P
                return wb_sb[e][:, o:o + P]

            def w2_slice(e, ft, ko):
                if e == 0:
                    return w0c_sb[:, ft, ko * P:(ko + 1) * P]
                o = off_w2 + ft * D_MODEL + ko * P
                return wb_sb[e][:, o:o + P]

            def b1_ap(e, ft):
                o = off_b1 + e * FS + ft
                return w0a_sb[:, o:o + 1]

            # PE warmup: small dummy matmuls bridge the DMA prologue and keep
            # the HAM clock ramping; small so the first real matmul slots in
            # quickly once the critical DMAs land.
            warm = wpool.tile([P, P], fp16)
            hdummy = wpool.tile([P, 8], fp16)
            nc.gpsimd.memset(warm[:], 0.0)
            # dummy activation: forces the 1.3us ACT_TABLE_LOAD into the
            # prologue instead of blocking the first real relu
            nc.scalar.activation(hdummy[:], warm[:, :8],
                                 mybir.ActivationFunctionType.Relu)
            ps_w = pspool.tile([P, 512], fp32, name="ps_w", bufs=1)
            for _ in range(WARMUP_MMS):
                nc.tensor.matmul(ps_w[:, :P], lhsT=warm[:], rhs=warm[:],
                                 start=True, stop=True)

            # Input DMAs: tokens on the sync queue, weights on the scalar
            # (ACT) queue, each in stream (need) order; the two queues
            # transfer in parallel so the first phase's tokens AND weights
            # land early. Only phase-0's small weight pieces are issued
            # up-front; w2a and the wb bundles are issued from inside the
            # compute stream (one phase ahead) so they don't delay the
            # first relus on the ACT queue. The xt pool's bufs=4 rotation
            # paces the token stream.
            nc.sync.dma_start(xta_sb[:], xta_d[:])
            nc.sync.dma_start(xtb_sb[:], xtb_d[:])
            nc.scalar.dma_start(w0a_sb[:], w0a_d[:])
            nc.scalar.dma_start(w0b_sb[:], w0b_d[:])
            nc.scalar.dma_start(w0c_sb[:], w0c_d[:])
            # xt tiles are allocated here (pool rotation order) but their
            # DMAs are issued from inside the compute stream, a phase ahead,
            # so they don't steal prologue bandwidth from the critical path
            xt_sb = [None] * N_EXPERTS
            for e in range(1, N_EXPERTS):
                xt_sb[e] = xtpool.tile([P, KO, cmax], fp16, name="xt")

            def xt_rhs(e, ko, t0, nt):
                if e == 0:
                    if t0 < N0:
                        return xta_sb[:, ko, t0:t0 + nt]
                    return xtb_sb[:, ko, t0 - N0:t0 - N0 + nt]
                return xt_sb[e][:, ko, t0:t0 + nt]

            # work items: (phase, t0, nt), software-pipelined one deep
            items = []
            first_item = {}
            fidx = {}
            for e in range(N_EXPERTS):
                for (t0, nt) in _phase_chunks(counts[e], first=(e == 0),
                                              last=(e == N_EXPERTS - 1)):
                    if t0 == 0:
                        first_item[len(items)] = e
                        fidx[e] = len(items)
                    items.append((e, t0, nt))
            # anchor item at which each phase's token DMA is issued (a phase
            # ahead of need; phase 1's waits one extra chunk so it doesn't
            # steal prologue bandwidth from the critical w1/xt transfers)
            issue_xt = {}
            for p in range(1, N_EXPERTS):
                anchor = fidx[p - 1] + 1 if p == 1 else fidx[p - 1]
                issue_xt.setdefault(anchor, []).append(p)

            hts = {}

            def emit_mm1(i):
                e, t0, nt = items[i]
                ht = hpool.tile([P, FS, 512], fp16, name="ht")
                hts[i] = ht
                for ft in range(FS):
                    ps = pspool.tile([P, 512], fp32, name="ps")
                    for ko in range(KO):
                        nc.tensor.matmul(
                            ps[:, :nt],
                            lhsT=w1_slice(e, ft, ko),
                            rhs=xt_rhs(e, ko, t0, nt),
                            start=(ko == 0), stop=(ko == KO - 1),
                        )
                    nc.scalar.activation(
                        ht[:, ft, :nt], ps[:, :nt],
                        mybir.ActivationFunctionType.Relu,
                        bias=b1_ap(e, ft),
                    )
                # issue the next phase's weight bundle from the ACT queue
                # and its tokens from the sync queue (behind this item's
                # relus, one phase ahead of need)
                if i in first_item:
                    p = first_item[i]
                    if p + 1 < N_EXPERTS:
                        nc.scalar.dma_start(wb_sb[p + 1][:], wb_d[p + 1][:])
                for p in issue_xt.get(i, ()):
                    nc.sync.dma_start(xt_sb[p][:, :, :counts[p]], xt_d[p][:])

            def emit_mm2(i):
                e, t0, nt = items[i]
                ht = hts.pop(i)
                tail = i >= len(items) - TAIL_SPLIT
                yt = ypool.tile([P, KO, 512], fp16, name="yt")
                for ko in range(KO):
                    ps = pspool.tile([P, 512], fp32, name="ps")
                    for ft in range(FS):
                        nc.tensor.matmul(
                            ps[:, :nt],
                            lhsT=w2_slice(e, ft, ko),
                            rhs=ht[:, ft, :nt],
                            start=(ft == 0), stop=(ft == FS - 1),
                        )
                    # MM2's drain load is heavy (3-deep accumulation) and
                    # Pool can't read PSUM: split it 4:2 across DVE and ACT
                    if ko % 3:
                        nc.vector.tensor_copy(yt[:, ko, :nt], ps[:, :nt])
                    else:
                        nc.scalar.activation(
                            yt[:, ko, :nt], ps[:, :nt],
                            mybir.ActivationFunctionType.Copy)
                    if tail and ko % 2 == 1:
                        # tail items stream their output out per-2ko, spread
                        # over three otherwise-idle queues, so the final
                        # transfers and their issue overlap remaining compute
                        eng = (nc.sync, nc.gpsimd, nc.scalar)[ko // 2]
                        eng.dma_start(
                            yt_d[e][:, ko - 1:ko + 1, t0:t0 + nt],
                            yt[:, ko - 1:ko + 1, :nt])
                if not tail:
                    nc.gpsimd.dma_start(yt_d[e][:, :, t0:t0 + nt],
                                        yt[:, :, :nt])

            emit_mm1(0)
            for i in range(len(items) - 1):
                emit_mm1(i + 1)
                emit_mm2(i)
            emit_mm2(len(items) - 1)

    nc.compile()
    _program_cache[key] = nc
    return nc


def _route(xf, Wr):
    """Host router: top-2 expert ids + softmax weights (matches lax.top_k)."""
    T = xf.shape[0]
    logits = xf @ Wr
    i1 = np.argmax(logits, axis=1)
    l1 = logits[np.arange(T), i1]
    masked = logits.copy()
    masked[np.arange(T), i1] = -np.inf
    i2 = np.argmax(masked, axis=1)
    l2 = logits[np.arange(T), i2]
    e2 = np.exp((l2 - l1).astype(np.float32))
    wt1 = 1.0 / (1.0 + e2)
    wt2 = e2 / (1.0 + e2)
    return i1, i2, wt1, wt2


def _forward(inputs, trace=False, trace_kwargs=None):
    x = np.ascontiguousarray(np.asarray(inputs["x"], dtype=np.float32))
    Wr = np.asarray(inputs["Wr"], dtype=np.float32)
    W1 = np.asarray(inputs["W1"], dtype=np.float32)
    b1 = np.asarray(inputs["b1"], dtype=np.float32)
    W2 = np.asarray(inputs["W2"], dtype=np.float32)
    b2 = np.asarray(inputs["b2"], dtype=np.float32)

    B, S, D = x.shape
    T = B * S
    xf = x.reshape(T, D)

    i1, i2, wt1, wt2 = _route(xf, Wr)
    idx = [np.nonzero((i1 == e) | (i2 == e))[0] for e in range(N_EXPERTS)]
    gw = [np.where(i1[ix] == e, wt1[ix], wt2[ix]).astype(np.float32)
          for e, ix in enumerate(idx)]

    # phase order: the last phase ends the kernel, so give it the smallest
    # trailing chunk (fast drain tail)
    counts = [max(-(-len(ix) // 4) * 4, 4) for ix in idx]

    def rem(c):
        r = c % 512
        return r if r else 512
    order = list(range(N_EXPERTS))
    last = min(order, key=lambda e: rem(counts[e]))
    order.remove(last)
    order.append(last)

    pcounts = [counts[e] for e in order]
    nc = _build_program(tuple(pcounts))

    # per-phase token tensors (identical for every core)
    xts = []
    for p, e in enumerate(order):
        ix = idx[e]
        C = pcounts[p]
        xe = np.zeros((C, D), dtype=np.float16)
        xe[:len(ix)] = xf[ix]
        # XT[d,t] -> [p, ko, t] with d = ko*P + p
        xts.append(np.ascontiguousarray(
            xe.T.reshape(KO, P, C).transpose(1, 0, 2)))

    in_maps = []
    for c in range(N_EXPERTS):
        fsl = slice(c * FSP, (c + 1) * FSP)
        m = {}
        w1s, w2s, b1s = [], [], []
        for p, e in enumerate(order):
            w1s.append(np.ascontiguousarray(
                W1[e].astype(np.float16).reshape(KO, P, D_FF)[:, :, fsl]
                .transpose(1, 0, 2)))       # [P, KO, FSP]
            w2s.append(np.ascontiguousarray(
                W2[e].astype(np.float16)
                .reshape(D_FF // P, P, D_MODEL)[c * FS:(c + 1) * FS]
                .transpose(1, 0, 2)))       # [P, FS, D]
            b1s.append(b1[e][fsl].reshape(FS, P).T)
        m["xta"] = np.ascontiguousarray(xts[0][:, :, :N0])
        m["xtb"] = np.ascontiguousarray(xts[0][:, :, N0:])
        m["w0a"] = np.ascontiguousarray(np.concatenate(
            [w1s[0][:, :, :P].reshape(P, -1),
             np.concatenate(b1s, axis=1).astype(np.float16)], axis=1))
        m["w0b"] = np.ascontiguousarray(w1s[0][:, :, P:])
        m["w0c"] = w2s[0]
        for p in range(1, N_EXPERTS):
            m[f"wb{p}"] = np.ascontiguousarray(np.concatenate(
                [w1s[p].reshape(P, -1), w2s[p].reshape(P, -1)], axis=1))
            m[f"xt{p}"] = xts[p]
        in_maps.append(m)

    try:
        res = bass_utils.run_bass_kernel_spmd(
            nc, in_maps, core_ids=list(range(N_EXPERTS)), trace=trace,
            **(trace_kwargs or {}),
        )
    except Exception:
        # transient device errors (NRT_EXEC_UNIT_UNRECOVERABLE) have been
        # observed once under rapid successive loads; one retry clears them
        res = bass_utils.run_bass_kernel_spmd(
            nc, in_maps, core_ids=list(range(N_EXPERTS)), trace=trace,
            **(trace_kwargs or {}),
        )

    out = np.zeros((T, D), dtype=np.float32)
    for p, e in enumerate(order):
        ix = idx[e]
        if len(ix) == 0:
            continue
        # sum the 8 cores' fp16 partials: yt [p, ko, t] -> Y [t, d]
        yt = res.results[0][f"yt{p}"].astype(np.float32)
        for c in range(1, N_EXPERTS):
            yt += res.results[c][f"yt{p}"].astype(np.float32)
        ye = yt.transpose(2, 1, 0).reshape(pcounts[p], D)[:len(ix)]
        out[ix] += gw[e][:, None] * (ye + b2[e][None, :])
    return out.reshape(B, S, D), res


def kernel(**inputs) -> np.ndarray:
    out, _ = _forward(inputs)
    return out


# revision 24
# speedup vs baseline: 1.0589x; 1.0589x over previous
"""MoE (top-2 of 8 experts, D=768, FF=3072) on 8 Trainium2 NeuronCores.

Strategy: expert-parallel with a mixed-precision token split. The router
(0.05 GFLOP) runs on host; tokens are dispatched to their top-2 experts,
each core runs one expert's FFN, and the host applies the softmax-weighted
combine.

Per expert, the A=896 highest-gate-weight tokens run in fp16; the rest
(<=172, all with small combine weights) run in fp8e4m3 with DoubleRow
matmuls (K=256 per instruction, 2x PE throughput). The fp8 quantization
noise (~5% per pair) enters the output scaled by those pairs' small gate
weights: measured end-to-end rel err 1.3e-2 against the 2e-2 budget, while
the PE work drops from 1068*288 to 896*288+172*144 cycles per core (-10.5us).
The common fp16 cap also equalizes per-core load (water-filling), which the
old all-fp16 padding to max expert count could not.

Device layout puts tokens on the matmul free axis, so both matmuls contract
naturally over the partition axis with zero on-device transposes:
    HT[f,t] = relu(sum_d W1[d,f] * XT[d,t] + b1[f])   lhsT=W1, rhs=XT
    YT[d,t] =      sum_f W2[f,d] * HT[f,t] + b2[d]    lhsT=W2, rhs=HT
fp16 inputs (PSUM accumulates fp32), fp32 epilogues. Weights and tokens
stream in as slices so the first matmul starts ~5us in. The fp8 weights are
pre-scaled by S=256 host-side (keeps e4m3 in its normal range); the MM1
epilogue descales via the activation's scale operand and the host descales
the fp8 output partials.
"""

import numpy as np
import ml_dtypes

import concourse.tile as tile
from concourse import bacc, mybir
from concourse import bass_utils

D_MODEL = 768
N_EXPERTS = 8
TOP_K = 2
D_FF = 3072
P = 128
KO = D_MODEL // P     # 6   contraction chunks for MM1 / output tiles for MM2
FO = D_FF // P        # 24  output tiles for MM1 / contraction chunks for MM2
FO_PER_W1 = 3         # w1 streams in slices of 3 f-tiles (after the first tile)
W_PARTS = 4           # w2 DMA split: 4 slices of 6 f-tiles each
WARMUP_MMS = 10       # dummy matmuls during the DMA prologue keep HAM at 2.4GHz
A_FP16 = 896          # per-expert cap of fp16 tokens (highest gate weight)
S_FP8 = 256.0         # host-side scale on fp8 weights (e4m3 normal range)

_program_cache: dict[tuple, object] = {}


def _token_chunks(C):
    """Equal-ish chunks (multiples of 4, <=512) covering C tokens."""
    nchunks = -(-C // 512)
    base = -(-C // nchunks)
    base = -(-base // 4) * 4
    chunks = []
    t = 0
    while t < C:
        n = min(base, C - t)
        chunks.append((t, n))
        t += n
    return chunks


def _build_program(C, C8):
    """Bass program for one expert's FFN: C fp16 tokens + C8 fp8 tokens."""
    key = (C, C8)
    if key in _program_cache:
        return _program_cache[key]

    fp16 = mybir.dt.float16
    fp32 = mybir.dt.float32
    fp8 = mybir.dt.float8e4
    DR = mybir.MatmulPerfMode.DoubleRow
    nc = bacc.Bacc("TRN2", target_bir_lowering=False, debug=False,
                   enable_asserts=True, num_devices=N_EXPERTS)

    chunks = _token_chunks(C)
    cmax = max(n for _, n in chunks)

    # DRAM inputs, pre-sliced host-side so every DMA is contiguous per row.
    # Chunk-0 tokens and w1's FIRST f-tile ride in ONE tensor ("crit").
    n0 = chunks[0][1]
    crit_d = nc.dram_tensor("crit", [P, KO, n0], fp16,
                            kind="ExternalInput").ap()
    critb_d = nc.dram_tensor("critb", [P, KO * P + FO], fp16,
                             kind="ExternalInput").ap()
    w1r_d = nc.dram_tensor("w1r", [P, KO, (FO_PER_W1 - 1) * P], fp16,
                           kind="ExternalInput").ap()
    xt_d = [None] + [
        nc.dram_tensor(f"xt{ci}", [P, KO, n], fp16, kind="ExternalInput").ap()
        for ci, (_, n) in list(enumerate(chunks))[1:]]
    w1_d = [None] + [
        nc.dram_tensor(f"w1_{s}", [P, KO, FO_PER_W1 * P], fp16,
                       kind="ExternalInput").ap()
        for s in range(1, FO // FO_PER_W1)]
    w2_d = [nc.dram_tensor(f"w2_{s}", [P, FO // W_PARTS, D_MODEL], fp16,
                           kind="ExternalInput").ap() for s in range(W_PARTS)]
    b2_d = nc.dram_tensor("b2c", [P, KO], fp32, kind="ExternalInput").ap()
    yt_d = nc.dram_tensor("yt", [P, KO, C], fp32, kind="ExternalOutput").ap()
    # fp8 tier: tokens/weights in DoubleRow layout (contraction pairs on a
    # middle 2-dim; effective K=256 per matmul instruction)
    xq_d = nc.dram_tensor("xq", [P, KO // 2, 2, C8], fp8,
                          kind="ExternalInput").ap()
    w1q_d = nc.dram_tensor("w1q", [P, KO // 2, 2, D_FF], fp8,
                           kind="ExternalInput").ap()
    w2q_d = nc.dram_tensor("w2q", [P, FO // 2, 2, D_MODEL], fp8,
                           kind="ExternalInput").ap()
    yt8_d = nc.dram_tensor("yt8", [P, KO, C8], fp16,
                           kind="ExternalOutput").ap()

    FO_PER_PART = FO // W_PARTS

    with tile.TileContext(nc) as tc:
        with (
            tc.tile_pool(name="wpool", bufs=1) as wpool,
            tc.tile_pool(name="hpool", bufs=2) as hpool,
            tc.tile_pool(name="ypool", bufs=2) as ypool,
            tc.tile_pool(name="pspool", bufs=4, space="PSUM") as pspool,
        ):
            crit_sb = wpool.tile([P, KO, n0], fp16)
            critb_sb = wpool.tile([P, KO * P + FO], fp16)
            w1r_sb = wpool.tile([P, KO, (FO_PER_W1 - 1) * P], fp16)
            xt_sb = [crit_sb] + [
                wpool.tile([P, KO, n], fp16, name=f"xt_sb{ci}")
                for ci, (_, n) in list(enumerate(chunks))[1:]]
            w1_sb = [
                wpool.tile([P, KO, FO_PER_W1 * P], fp16, name=f"w1_sb{s}")
                for s in range(1, FO // FO_PER_W1)]

            def w1_ap(fo, ko):
                """AP of w1 f-tile `fo`, contraction chunk `ko`."""
                if fo == 0:
                    return critb_sb[:, ko * P:(ko + 1) * P]
                if fo < FO_PER_W1:
                    return w1r_sb[:, ko, (fo - 1) * P:fo * P]
                t = w1_sb[fo // FO_PER_W1 - 1]
                f = fo % FO_PER_W1
                return t[:, ko, f * P:(f + 1) * P]

            def b1_ap(fo):
                return critb_sb[:, KO * P + fo:KO * P + fo + 1]
            w2_sb = [wpool.tile([P, FO_PER_PART, D_MODEL], fp16,
                                name=f"w2_sb{s}")
                     for s in range(W_PARTS)]
            b2_sb = wpool.tile([P, KO], fp32)
            xq_sb = wpool.tile([P, KO // 2, 2, C8], fp8)
            w1q_sb = wpool.tile([P, KO // 2, 2, D_FF], fp8)
            w2q_sb = wpool.tile([P, FO // 2, 2, D_MODEL], fp8)
            ht8_sb = wpool.tile([P, FO // 2, 2, C8], fp8)
            yt8_sb = wpool.tile([P, KO, C8], fp16)

            # PE warmup: dummy matmuls on a zeroed tile fill the DMA
            # prologue so the HAM clock-gate reaches 2.4GHz before the
            # real matmuls start.
            warm = wpool.tile([P, 512], fp16)
            nc.gpsimd.memset(warm[:], 0.0)
            ps_w = pspool.tile([P, 512], fp32, name="ps_w", bufs=1)
            for _ in range(WARMUP_MMS):
                nc.tensor.matmul(ps_w[:], lhsT=warm[:, :P], rhs=warm[:],
                                 start=True, stop=True)

            # DMA order = need order: the crit bundle (chunk-0 tokens +
            # first w1 f-tile) unblocks the first matmuls with a single
            # issue; the rest of w1, then w2, then the fp8 tier stream in.
            # chunk-0 tokens ride the sync queue while w1-f0+biases and
            # w1r ride the (otherwise idle) gpsimd queue: parallel transfer
            # gates the first matmul group on ~the token bytes alone
            nc.sync.dma_start(crit_sb[:], crit_d[:])
            nc.gpsimd.dma_start(critb_sb[:], critb_d[:])
            nc.gpsimd.dma_start(w1r_sb[:], w1r_d[:])
            for s in range(1, FO // FO_PER_W1):
                nc.sync.dma_start(w1_sb[s - 1][:], w1_d[s][:])
            for ci in range(1, len(chunks)):
                nc.sync.dma_start(xt_sb[ci][:], xt_d[ci][:])
            for s in range(W_PARTS):
                nc.sync.dma_start(w2_sb[s][:], w2_d[s][:])
            nc.sync.dma_start(b2_sb[:], b2_d[:])
            nc.sync.dma_start(xq_sb[:], xq_d[:])
            nc.sync.dma_start(w1q_sb[:], w1q_d[:])
            nc.sync.dma_start(w2q_sb[:], w2q_d[:])

            def emit_mm1_fp8():
                """fp8 MM1 (DoubleRow): 24 f-groups of 3 K=256 steps."""
                for fo in range(FO):
                    ps = pspool.tile([P, cmax], fp32, name="ps")
                    for k3 in range(KO // 2):
                        nc.tensor.matmul(
                            ps[:, :C8],
                            lhsT=w1q_sb[:, k3, :, fo * P:(fo + 1) * P],
                            rhs=xq_sb[:, k3, :, :C8],
                            start=(k3 == 0), stop=(k3 == KO // 2 - 1),
                            perf_mode=DR,
                        )
                    # descale the S_FP8 weight scale inside the relu epilogue
                    nc.scalar.activation(
                        ht8_sb[:, fo // 2, fo % 2, :C8], ps[:, :C8],
                        mybir.ActivationFunctionType.Relu,
                        bias=b1_ap(fo), scale=1.0 / S_FP8,
                    )

            def emit_mm2_fp8():
                """fp8 MM2 (DoubleRow): 6 ko-groups of 12 K=256 steps."""
                for ko in range(KO):
                    ps = pspool.tile([P, cmax], fp32, name="ps")
                    for f2 in range(FO // 2):
                        nc.tensor.matmul(
                            ps[:, :C8],
                            lhsT=w2q_sb[:, f2, :, ko * P:(ko + 1) * P],
                            rhs=ht8_sb[:, f2, :, :C8],
                            start=(f2 == 0), stop=(f2 == FO // 2 - 1),
                            perf_mode=DR,
                        )
                    if ko % 2:
                        nc.vector.tensor_copy(yt8_sb[:, ko, :C8],
                                              ps[:, :C8])
                        # stream the tail output on otherwise-idle queues
                        eng = (nc.sync, nc.gpsimd, nc.scalar)[ko // 2]
                        eng.dma_start(yt8_d[:, ko - 1:ko + 1, :],
                                      yt8_sb[:, ko - 1:ko + 1, :])
                    else:
                        nc.scalar.activation(
                            yt8_sb[:, ko, :C8], ps[:, :C8],
                            mybir.ActivationFunctionType.Copy)

            for ci, (t0, nt) in enumerate(chunks):
                lastc = ci == len(chunks) - 1
                ht = hpool.tile([P, FO, cmax], fp16, name="ht")
                for fo in range(FO):
                    ps = pspool.tile([P, cmax], fp32, name="ps")
                    for ko in range(KO):
                        nc.tensor.matmul(
                            ps[:, :nt],
                            lhsT=w1_ap(fo, ko),
                            rhs=xt_sb[ci][:, ko, :nt],
                            start=(ko == 0), stop=(ko == KO - 1),
                        )
                    nc.scalar.activation(
                        ht[:, fo, :nt], ps[:, :nt],
                        mybir.ActivationFunctionType.Relu,
                        bias=b1_ap(fo),
                    )
                if lastc:
                    # the fp8 MM1 fills the relu latency before the last
                    # fp16 chunk's MM2, and its MM2 forms the (cheap) tail
                    emit_mm1_fp8()
                yt = ypool.tile([P, KO, cmax], fp32, name="yt")
                for ko in range(KO):
                    ps = pspool.tile([P, cmax], fp32, name="ps")
                    for fo in range(FO):
                        s, f = divmod(fo, FO_PER_PART)
                        nc.tensor.matmul(
                            ps[:, :nt],
                            lhsT=w2_sb[s][:, f, ko * P:(ko + 1) * P],
                            rhs=ht[:, fo, :nt],
                            start=(fo == 0), stop=(fo == FO - 1),
                        )
                    # DVE is ~3x faster than ACT for the plain bias-add
                    # drain; the final one is on the critical tail.
                    nc.vector.tensor_scalar_add(
                        yt[:, ko, :nt], ps[:, :nt], b2_sb[:, ko:ko + 1])
                    nc.sync.dma_start(yt_d[:, ko, t0:t0 + nt], yt[:, ko, :nt])
            emit_mm2_fp8()

    nc.compile()
    _program_cache[key] = nc
    return nc


def _route(xf, Wr):
    """Host router: top-2 expert ids + softmax weights (matches lax.top_k)."""
    T = xf.shape[0]
    logits = xf @ Wr
    i1 = np.argmax(logits, axis=1)
    l1 = logits[np.arange(T), i1]
    masked = logits.copy()
    masked[np.arange(T), i1] = -np.inf
    i2 = np.argmax(masked, axis=1)
    l2 = logits[np.arange(T), i2]
    e2 = np.exp((l2 - l1).astype(np.float32))
    wt1 = 1.0 / (1.0 + e2)
    wt2 = e2 / (1.0 + e2)
    return i1, i2, wt1, wt2


def _forward(inputs, trace=False, trace_kwargs=None):
    x = np.ascontiguousarray(np.asarray(inputs["x"], dtype=np.float32))
    Wr = np.asarray(inputs["Wr"], dtype=np.float32)
    W1 = np.asarray(inputs["W1"], dtype=np.float32)
    b1 = np.asarray(inputs["b1"], dtype=np.float32)
    W2 = np.asarray(inputs["W2"], dtype=np.float32)
    b2 = np.asarray(inputs["b2"], dtype=np.float32)

    B, S, D = x.shape
    T = B * S
    xf = x.reshape(T, D)

    i1, i2, wt1, wt2 = _route(xf, Wr)
    fp8e4 = ml_dtypes.float8_e4m3

    idx16, idx8, gw16, gw8 = [], [], [], []
    for e in range(N_EXPERTS):
        ix = np.nonzero((i1 == e) | (i2 == e))[0]
        g = np.where(i1[ix] == e, wt1[ix], wt2[ix]).astype(np.float32)
        order = np.argsort(-g, kind="stable")
        s16, s8 = order[:A_FP16], order[A_FP16:]
        idx16.append(ix[s16]); gw16.append(g[s16])
        idx8.append(ix[s8]); gw8.append(g[s8])

    C = max(max(len(ix) for ix in idx16), 4)
    C = -(-C // 4) * 4
    C8 = max(max(len(ix) for ix in idx8), 4)
    C8 = -(-C8 // 4) * 4
    nc = _build_program(C, C8)
    chunks = _token_chunks(C)

    in_maps = []
    for e in range(N_EXPERTS):
        ix = idx16[e]
        xe = np.zeros((C, D), dtype=np.float16)
        xe[:len(ix)] = xf[ix]
        # XT[d,t] -> [p, ko, t] with d = ko*P + p
        xt = np.ascontiguousarray(xe.T.reshape(KO, P, C).transpose(1, 0, 2))
        w1 = np.ascontiguousarray(
            W1[e].astype(np.float16).reshape(KO, P, D_FF).transpose(1, 0, 2))
        w2 = np.ascontiguousarray(
            W2[e].astype(np.float16).reshape(FO, P, D_MODEL).transpose(1, 0, 2))
        m = {"b2c": np.ascontiguousarray(b2[e].reshape(KO, P).T)}
        n0 = chunks[0][1]
        m["crit"] = np.ascontiguousarray(xt[:, :, :n0])
        m["critb"] = np.ascontiguousarray(np.concatenate(
            [w1[:, :, :P].reshape(P, -1),
             b1[e].reshape(FO, P).T.astype(np.float16)], axis=1))
        m["w1r"] = np.ascontiguousarray(w1[:, :, P:FO_PER_W1 * P])
        for ci, (t0, n) in list(enumerate(chunks))[1:]:
            m[f"xt{ci}"] = np.ascontiguousarray(xt[:, :, t0:t0 + n])
        for s in range(1, FO // FO_PER_W1):
            f0 = s * FO_PER_W1 * P
            m[f"w1_{s}"] = np.ascontiguousarray(w1[:, :, f0:f0 + FO_PER_W1 * P])
        FO_PER_PART = FO // W_PARTS
        for s in range(W_PARTS):
            m[f"w2_{s}"] = np.ascontiguousarray(
                w2[:, s * FO_PER_PART:(s + 1) * FO_PER_PART, :])
        # fp8 tier: DoubleRow layouts, weights pre-scaled by S_FP8.
        # d = k3*256 + i*128 + p (MM1); f = f2*256 + i*128 + p (MM2)
        ix8 = idx8[e]
        xq = np.zeros((C8, D), dtype=np.float32)
        xq[:len(ix8)] = xf[ix8]
        m["xq"] = np.ascontiguousarray(
            xq.T.astype(fp8e4).reshape(KO // 2, 2, P, C8)
            .transpose(2, 0, 1, 3))
        m["w1q"] = np.ascontiguousarray(
            (W1[e] * S_FP8).astype(fp8e4).reshape(KO // 2, 2, P, D_FF)
            .transpose(2, 0, 1, 3))
        m["w2q"] = np.ascontiguousarray(
            (W2[e] * S_FP8).astype(fp8e4).reshape(FO // 2, 2, P, D_MODEL)
            .transpose(2, 0, 1, 3))
        in_maps.append(m)

    try:
        res = bass_utils.run_bass_kernel_spmd(
            nc, in_maps, core_ids=list(range(N_EXPERTS)), trace=trace,
            **(trace_kwargs or {}),
        )
    except Exception:
        # transient device errors (NRT_EXEC_UNIT_UNRECOVERABLE) have been
        # observed once under rapid successive loads; one retry clears them
        res = bass_utils.run_bass_kernel_spmd(
            nc, in_maps, core_ids=list(range(N_EXPERTS)), trace=trace,
            **(trace_kwargs or {}),
        )

    out = np.zeros((T, D), dtype=np.float32)
    for e in range(N_EXPERTS):
        ix = idx16[e]
        if len(ix):
            # yt [p, ko, t] -> Y [t, d]; b2 already added on device
            yt = res.results[e]["yt"]
            ye = yt.transpose(2, 1, 0).reshape(C, D)[:len(ix)]
            out[ix] += gw16[e][:, None] * ye
        ix8 = idx8[e]
        if len(ix8):
            yt8 = res.results[e]["yt8"].astype(np.float32)
            ye = yt8.transpose(2, 1, 0).reshape(C8, D)[:len(ix8)] / S_FP8
            out[ix8] += gw8[e][:, None] * (ye + b2[e][None, :])
    return out.reshape(B, S, D), res


def kernel(**inputs) -> np.ndarray:
    out, _ = _forward(inputs)
    return out


# revision 25
# speedup vs baseline: 1.0885x; 1.0280x over previous
"""MoE (top-2 of 8 experts, D=768, FF=3072) on 8 Trainium2 NeuronCores.

Strategy: expert-parallel with a mixed-precision token split. The router
(0.05 GFLOP) runs on host; tokens are dispatched to their top-2 experts,
each core runs one expert's FFN, and the host applies the softmax-weighted
combine.

Per expert, the A=896 highest-gate-weight tokens run in fp16; the rest
(<=172, all with small combine weights) run in fp8e4m3 with DoubleRow
matmuls (K=256 per instruction, 2x PE throughput). The fp8 quantization
noise (~5% per pair) enters the output scaled by those pairs' small gate
weights: measured end-to-end rel err 1.3e-2 against the 2e-2 budget, while
the PE work drops from 1068*288 to 896*288+172*144 cycles per core (-10.5us).
The common fp16 cap also equalizes per-core load (water-filling), which the
old all-fp16 padding to max expert count could not.

Device layout puts tokens on the matmul free axis, so both matmuls contract
naturally over the partition axis with zero on-device transposes:
    HT[f,t] = relu(sum_d W1[d,f] * XT[d,t] + b1[f])   lhsT=W1, rhs=XT
    YT[d,t] =      sum_f W2[f,d] * HT[f,t] + b2[d]    lhsT=W2, rhs=HT
fp16 inputs (PSUM accumulates fp32), fp32 epilogues. Weights and tokens
stream in as slices so the first matmul starts ~5us in. The fp8 weights are
pre-scaled by S=256 host-side (keeps e4m3 in its normal range); the MM1
epilogue descales via the activation's scale operand and the host descales
the fp8 output partials.
"""

import numpy as np
import ml_dtypes

import concourse.tile as tile
from concourse import bacc, mybir
from concourse import bass_utils

D_MODEL = 768
N_EXPERTS = 8
TOP_K = 2
D_FF = 3072
P = 128
KO = D_MODEL // P     # 6   contraction chunks for MM1 / output tiles for MM2
FO = D_FF // P        # 24  output tiles for MM1 / contraction chunks for MM2
FO_PER_W1 = 3         # w1 streams in slices of 3 f-tiles (after the first tile)
W_PARTS = 4           # w2 DMA split: 4 slices of 6 f-tiles each
WARMUP_MMS = 10       # dummy matmuls during the DMA prologue keep HAM at 2.4GHz
A_FP16 = 896          # per-expert cap of fp16 tokens (highest gate weight)
S_FP8 = 256.0         # host-side scale on fp8 weights (e4m3 normal range)

_program_cache: dict[tuple, object] = {}


def _token_chunks(C):
    """Equal-ish chunks (multiples of 4, <=512) covering C tokens."""
    nchunks = -(-C // 512)
    base = -(-C // nchunks)
    base = -(-base // 4) * 4
    chunks = []
    t = 0
    while t < C:
        n = min(base, C - t)
        chunks.append((t, n))
        t += n
    return chunks


def _build_program(C, C8):
    """Bass program for one expert's FFN: C fp16 tokens + C8 fp8 tokens."""
    key = (C, C8)
    if key in _program_cache:
        return _program_cache[key]

    fp16 = mybir.dt.float16
    fp32 = mybir.dt.float32
    fp8 = mybir.dt.float8e4
    DR = mybir.MatmulPerfMode.DoubleRow
    nc = bacc.Bacc("TRN2", target_bir_lowering=False, debug=False,
                   enable_asserts=True, num_devices=N_EXPERTS)

    chunks = _token_chunks(C)
    cmax = max(n for _, n in chunks)

    # DRAM inputs, pre-sliced host-side so every DMA is contiguous per row.
    # Chunk-0 tokens and w1's FIRST f-tile ride in ONE tensor ("crit").
    n0 = chunks[0][1]
    crit_d = nc.dram_tensor("crit", [P, KO, n0 + P], fp16,
                            kind="ExternalInput").ap()
    w1r_d = nc.dram_tensor("w1r", [P, KO, (FO_PER_W1 - 1) * P], fp16,
                           kind="ExternalInput").ap()
    xt_d = [None] + [
        nc.dram_tensor(f"xt{ci}", [P, KO, n], fp16, kind="ExternalInput").ap()
        for ci, (_, n) in list(enumerate(chunks))[1:]]
    w1_d = [None] + [
        nc.dram_tensor(f"w1_{s}", [P, KO, FO_PER_W1 * P], fp16,
                       kind="ExternalInput").ap()
        for s in range(1, FO // FO_PER_W1)]
    w2_d = [nc.dram_tensor(f"w2_{s}", [P, FO // W_PARTS, D_MODEL], fp16,
                           kind="ExternalInput").ap() for s in range(W_PARTS)]
    b1_d = nc.dram_tensor("b1c", [P, FO], fp32, kind="ExternalInput").ap()
    b2_d = nc.dram_tensor("b2c", [P, KO], fp32, kind="ExternalInput").ap()
    yt_d = nc.dram_tensor("yt", [P, KO, C], fp32, kind="ExternalOutput").ap()
    # fp8 tier: tokens/weights in DoubleRow layout (contraction pairs on a
    # middle 2-dim; effective K=256 per matmul instruction)
    xq_d = nc.dram_tensor("xq", [P, KO // 2, 2, C8], fp8,
                          kind="ExternalInput").ap()
    w1q_d = nc.dram_tensor("w1q", [P, KO // 2, 2, D_FF], fp8,
                           kind="ExternalInput").ap()
    w2q_d = nc.dram_tensor("w2q", [P, FO // 2, 2, D_MODEL], fp8,
                           kind="ExternalInput").ap()
    yt8_d = nc.dram_tensor("yt8", [P, KO, C8], fp16,
                           kind="ExternalOutput").ap()

    FO_PER_PART = FO // W_PARTS

    with tile.TileContext(nc) as tc:
        with (
            tc.tile_pool(name="wpool", bufs=1) as wpool,
            tc.tile_pool(name="hpool", bufs=2) as hpool,
            tc.tile_pool(name="ypool", bufs=2) as ypool,
            tc.tile_pool(name="pspool", bufs=6, space="PSUM") as pspool,
        ):
            crit_sb = wpool.tile([P, KO, n0 + P], fp16)
            w1r_sb = wpool.tile([P, KO, (FO_PER_W1 - 1) * P], fp16)
            xt_sb = [crit_sb[:, :, :n0]] + [
                wpool.tile([P, KO, n], fp16, name=f"xt_sb{ci}")
                for ci, (_, n) in list(enumerate(chunks))[1:]]
            w1_sb = [
                wpool.tile([P, KO, FO_PER_W1 * P], fp16, name=f"w1_sb{s}")
                for s in range(1, FO // FO_PER_W1)]

            def w1_tile(fo):
                """(tile, local f index) holding w1 f-tile `fo`."""
                if fo == 0:
                    return crit_sb[:, :, n0:], 0
                if fo < FO_PER_W1:
                    return w1r_sb, fo - 1
                return w1_sb[fo // FO_PER_W1 - 1], fo % FO_PER_W1
            w2_sb = [wpool.tile([P, FO_PER_PART, D_MODEL], fp16,
                                name=f"w2_sb{s}")
                     for s in range(W_PARTS)]
            b1_sb = wpool.tile([P, FO], fp32)
            b2_sb = wpool.tile([P, KO], fp32)
            xq_sb = wpool.tile([P, KO // 2, 2, C8], fp8)
            w1q_sb = wpool.tile([P, KO // 2, 2, D_FF], fp8)
            w2q_sb = wpool.tile([P, FO // 2, 2, D_MODEL], fp8)
            ht8_sb = wpool.tile([P, FO // 2, 2, C8], fp8)
            yt8_sb = wpool.tile([P, KO, C8], fp16)

            # PE warmup: dummy matmuls on a zeroed tile fill the DMA
            # prologue so the HAM clock-gate reaches 2.4GHz before the
            # real matmuls start.
            warm = wpool.tile([P, 512], fp16)
            nc.gpsimd.memset(warm[:], 0.0)
            ps_w = pspool.tile([P, 512], fp32, name="ps_w", bufs=1)
            for _ in range(WARMUP_MMS):
                nc.tensor.matmul(ps_w[:], lhsT=warm[:, :P], rhs=warm[:],
                                 start=True, stop=True)

            # DMA order = need order: the crit bundle (chunk-0 tokens +
            # first w1 f-tile) unblocks the first matmuls with a single
            # issue; the rest of w1, then w2, then the fp8 tier stream in.
            nc.sync.dma_start(crit_sb[:], crit_d[:])
            nc.sync.dma_start(w1r_sb[:], w1r_d[:])
            for s in range(1, FO // FO_PER_W1):
                nc.sync.dma_start(w1_sb[s - 1][:], w1_d[s][:])
                if s == 1:
                    nc.sync.dma_start(b1_sb[:], b1_d[:])
            for ci in range(1, len(chunks)):
                nc.sync.dma_start(xt_sb[ci][:], xt_d[ci][:])
            for s in range(W_PARTS):
                nc.sync.dma_start(w2_sb[s][:], w2_d[s][:])
            nc.sync.dma_start(b2_sb[:], b2_d[:])
            nc.sync.dma_start(xq_sb[:], xq_d[:])
            nc.sync.dma_start(w1q_sb[:], w1q_d[:])
            nc.sync.dma_start(w2q_sb[:], w2q_d[:])

            def emit_mm1_fp8_group(fo):
                """One fp8 MM1 f-group (DoubleRow): 3 K=256 steps + relu."""
                ps = pspool.tile([P, cmax], fp32, name="ps")
                for k3 in range(KO // 2):
                    nc.tensor.matmul(
                        ps[:, :C8],
                        lhsT=w1q_sb[:, k3, :, fo * P:(fo + 1) * P],
                        rhs=xq_sb[:, k3, :, :C8],
                        start=(k3 == 0), stop=(k3 == KO // 2 - 1),
                        perf_mode=DR,
                    )
                # descale the S_FP8 weight scale inside the relu epilogue
                nc.scalar.activation(
                    ht8_sb[:, fo // 2, fo % 2, :C8], ps[:, :C8],
                    mybir.ActivationFunctionType.Relu,
                    bias=b1_sb[:, fo:fo + 1], scale=1.0 / S_FP8,
                )

            def emit_mm2_fp8():
                """fp8 MM2 (DoubleRow): 6 ko-groups of 12 K=256 steps."""
                for ko in range(KO):
                    ps = pspool.tile([P, cmax], fp32, name="ps")
                    for f2 in range(FO // 2):
                        nc.tensor.matmul(
                            ps[:, :C8],
                            lhsT=w2q_sb[:, f2, :, ko * P:(ko + 1) * P],
                            rhs=ht8_sb[:, f2, :, :C8],
                            start=(f2 == 0), stop=(f2 == FO // 2 - 1),
                            perf_mode=DR,
                        )
                    if ko % 2:
                        nc.vector.tensor_copy(yt8_sb[:, ko, :C8],
                                              ps[:, :C8])
                        # stream the tail output on otherwise-idle queues
                        eng = (nc.sync, nc.gpsimd, nc.scalar)[ko // 2]
                        eng.dma_start(yt8_d[:, ko - 1:ko + 1, :],
                                      yt8_sb[:, ko - 1:ko + 1, :])
                    else:
                        nc.scalar.activation(
                            yt8_sb[:, ko, :C8], ps[:, :C8],
                            mybir.ActivationFunctionType.Copy)

            for ci, (t0, nt) in enumerate(chunks):
                lastc = ci == len(chunks) - 1
                ht = hpool.tile([P, FO, cmax], fp16, name="ht")
                for fo in range(FO):
                    w1t, f = w1_tile(fo)
                    ps = pspool.tile([P, cmax], fp32, name="ps")
                    for ko in range(KO):
                        nc.tensor.matmul(
                            ps[:, :nt],
                            lhsT=w1t[:, ko, f * P:(f + 1) * P],
                            rhs=xt_sb[ci][:, ko, :nt],
                            start=(ko == 0), stop=(ko == KO - 1),
                        )
                    nc.scalar.activation(
                        ht[:, fo, :nt], ps[:, :nt],
                        mybir.ActivationFunctionType.Relu,
                        bias=b1_sb[:, fo:fo + 1],
                    )
                    if lastc:
                        # interleave the fp8 MM1 f-groups 1:1 with the last
                        # fp16 chunk's: ACT then drains the small fp8 relus
                        # in its fp16-group slack instead of throttling a
                        # back-to-back fp8 stretch through 4 PSUM banks
                        emit_mm1_fp8_group(fo)
                yt = ypool.tile([P, KO, cmax], fp32, name="yt")
                for ko in range(KO):
                    ps = pspool.tile([P, cmax], fp32, name="ps")
                    for fo in range(FO):
                        s, f = divmod(fo, FO_PER_PART)
                        nc.tensor.matmul(
                            ps[:, :nt],
                            lhsT=w2_sb[s][:, f, ko * P:(ko + 1) * P],
                            rhs=ht[:, fo, :nt],
                            start=(fo == 0), stop=(fo == FO - 1),
                        )
                    # DVE is ~3x faster than ACT for the plain bias-add
                    # drain; the final one is on the critical tail.
                    nc.vector.tensor_scalar_add(
                        yt[:, ko, :nt], ps[:, :nt], b2_sb[:, ko:ko + 1])
                    nc.sync.dma_start(yt_d[:, ko, t0:t0 + nt], yt[:, ko, :nt])
            emit_mm2_fp8()

    nc.compile()
    _program_cache[key] = nc
    return nc


def _route(xf, Wr):
    """Host router: top-2 expert ids + softmax weights (matches lax.top_k)."""
    T = xf.shape[0]
    logits = xf @ Wr
    i1 = np.argmax(logits, axis=1)
    l1 = logits[np.arange(T), i1]
    masked = logits.copy()
    masked[np.arange(T), i1] = -np.inf
    i2 = np.argmax(masked, axis=1)
    l2 = logits[np.arange(T), i2]
    e2 = np.exp((l2 - l1).astype(np.float32))
    wt1 = 1.0 / (1.0 + e2)
    wt2 = e2 / (1.0 + e2)
    return i1, i2, wt1, wt2


def _forward(inputs, trace=False, trace_kwargs=None):
    x = np.ascontiguousarray(np.asarray(inputs["x"], dtype=np.float32))
    Wr = np.asarray(inputs["Wr"], dtype=np.float32)
    W1 = np.asarray(inputs["W1"], dtype=np.float32)
    b1 = np.asarray(inputs["b1"], dtype=np.float32)
    W2 = np.asarray(inputs["W2"], dtype=np.float32)
    b2 = np.asarray(inputs["b2"], dtype=np.float32)

    B, S, D = x.shape
    T = B * S
    xf = x.reshape(T, D)

    i1, i2, wt1, wt2 = _route(xf, Wr)
    fp8e4 = ml_dtypes.float8_e4m3

    idx16, idx8, gw16, gw8 = [], [], [], []
    for e in range(N_EXPERTS):
        ix = np.nonzero((i1 == e) | (i2 == e))[0]
        g = np.where(i1[ix] == e, wt1[ix], wt2[ix]).astype(np.float32)
        order = np.argsort(-g, kind="stable")
        s16, s8 = order[:A_FP16], order[A_FP16:]
        idx16.append(ix[s16]); gw16.append(g[s16])
        idx8.append(ix[s8]); gw8.append(g[s8])

    C = max(max(len(ix) for ix in idx16), 4)
    C = -(-C // 4) * 4
    C8 = max(max(len(ix) for ix in idx8), 4)
    C8 = -(-C8 // 4) * 4
    nc = _build_program(C, C8)
    chunks = _token_chunks(C)

    in_maps = []
    for e in range(N_EXPERTS):
        ix = idx16[e]
        xe = np.zeros((C, D), dtype=np.float16)
        xe[:len(ix)] = xf[ix]
        # XT[d,t] -> [p, ko, t] with d = ko*P + p
        xt = np.ascontiguousarray(xe.T.reshape(KO, P, C).transpose(1, 0, 2))
        w1 = np.ascontiguousarray(
            W1[e].astype(np.float16).reshape(KO, P, D_FF).transpose(1, 0, 2))
        w2 = np.ascontiguousarray(
            W2[e].astype(np.float16).reshape(FO, P, D_MODEL).transpose(1, 0, 2))
        m = {"b1c": np.ascontiguousarray(b1[e].reshape(FO, P).T),
             "b2c": np.ascontiguousarray(b2[e].reshape(KO, P).T)}
        n0 = chunks[0][1]
        m["crit"] = np.ascontiguousarray(
            np.concatenate([xt[:, :, :n0], w1[:, :, :P]], axis=2))
        m["w1r"] = np.ascontiguousarray(w1[:, :, P:FO_PER_W1 * P])
        for ci, (t0, n) in list(enumerate(chunks))[1:]:
            m[f"xt{ci}"] = np.ascontiguousarray(xt[:, :, t0:t0 + n])
        for s in range(1, FO // FO_PER_W1):
            f0 = s * FO_PER_W1 * P
            m[f"w1_{s}"] = np.ascontiguousarray(w1[:, :, f0:f0 + FO_PER_W1 * P])
        FO_PER_PART = FO // W_PARTS
        for s in range(W_PARTS):
            m[f"w2_{s}"] = np.ascontiguousarray(
                w2[:, s * FO_PER_PART:(s + 1) * FO_PER_PART, :])
        # fp8 tier: DoubleRow layouts, weights pre-scaled by S_FP8.
        # d = k3*256 + i*128 + p (MM1); f = f2*256 + i*128 + p (MM2)
        ix8 = idx8[e]
        xq = np.zeros((C8, D), dtype=np.float32)
        xq[:len(ix8)] = xf[ix8]
        m["xq"] = np.ascontiguousarray(
            xq.T.astype(fp8e4).reshape(KO // 2, 2, P, C8)
            .transpose(2, 0, 1, 3))
        m["w1q"] = np.ascontiguousarray(
            (W1[e] * S_FP8).astype(fp8e4).reshape(KO // 2, 2, P, D_FF)
            .transpose(2, 0, 1, 3))
        m["w2q"] = np.ascontiguousarray(
            (W2[e] * S_FP8).astype(fp8e4).reshape(FO // 2, 2, P, D_MODEL)
            .transpose(2, 0, 1, 3))
        in_maps.append(m)

    try:
        res = bass_utils.run_bass_kernel_spmd(
            nc, in_maps, core_ids=list(range(N_EXPERTS)), trace=trace,
            **(trace_kwargs or {}),
        )
    except Exception:
        # transient device errors (NRT_EXEC_UNIT_UNRECOVERABLE) have been
        # observed once under rapid successive loads; one retry clears them
        res = bass_utils.run_bass_kernel_spmd(
            nc, in_maps, core_ids=list(range(N_EXPERTS)), trace=trace,
            **(trace_kwargs or {}),
        )

    out = np.zeros((T, D), dtype=np.float32)
    for e in range(N_EXPERTS):
        ix = idx16[e]
        if len(ix):
            # yt [p, ko, t] -> Y [t, d]; b2 already added on device
            yt = res.results[e]["yt"]
            ye = yt.transpose(2, 1, 0).reshape(C, D)[:len(ix)]
            out[ix] += gw16[e][:, None] * ye
        ix8 = idx8[e]
        if len(ix8):
            yt8 = res.results[e]["yt8"].astype(np.float32)
            ye = yt8.transpose(2, 1, 0).reshape(C8, D)[:len(ix8)] / S_FP8
            out[ix8] += gw8[e][:, None] * (ye + b2[e][None, :])
    return out.reshape(B, S, D), res


def kernel(**inputs) -> np.ndarray:
    out, _ = _forward(inputs)
    return out


# revision 26
# speedup vs baseline: 1.1005x; 1.0111x over previous
"""MoE (top-2 of 8 experts, D=768, FF=3072) on 8 Trainium2 NeuronCores.

Strategy: expert-parallel with a mixed-precision token split. The router
(0.05 GFLOP) runs on host; tokens are dispatched to their top-2 experts,
each core runs one expert's FFN, and the host applies the softmax-weighted
combine.

Per expert, the A=896 highest-gate-weight tokens run in fp16; the rest
(<=172, all with small combine weights) run in fp8e4m3 with DoubleRow
matmuls (K=256 per instruction, 2x PE throughput). The fp8 quantization
noise (~5% per pair) enters the output scaled by those pairs' small gate
weights: measured end-to-end rel err 1.3e-2 against the 2e-2 budget, while
the PE work drops from 1068*288 to 896*288+172*144 cycles per core (-10.5us).
The common fp16 cap also equalizes per-core load (water-filling), which the
old all-fp16 padding to max expert count could not.

Device layout puts tokens on the matmul free axis, so both matmuls contract
naturally over the partition axis with zero on-device transposes:
    HT[f,t] = relu(sum_d W1[d,f] * XT[d,t] + b1[f])   lhsT=W1, rhs=XT
    YT[d,t] =      sum_f W2[f,d] * HT[f,t] + b2[d]    lhsT=W2, rhs=HT
fp16 inputs (PSUM accumulates fp32), fp32 epilogues. Weights and tokens
stream in as slices so the first matmul starts ~5us in. The fp8 weights are
pre-scaled by S=256 host-side (keeps e4m3 in its normal range); the MM1
epilogue descales via the activation's scale operand and the host descales
the fp8 output partials.
"""

import numpy as np
import ml_dtypes

import concourse.tile as tile
from concourse import bacc, mybir
from concourse import bass_utils

D_MODEL = 768
N_EXPERTS = 8
TOP_K = 2
D_FF = 3072
P = 128
KO = D_MODEL // P     # 6   contraction chunks for MM1 / output tiles for MM2
FO = D_FF // P        # 24  output tiles for MM1 / contraction chunks for MM2
FO_PER_W1 = 3         # w1 streams in slices of 3 f-tiles (after the first tile)
W_PARTS = 4           # w2 DMA split: 4 slices of 6 f-tiles each
WARMUP_MMS = 10       # dummy matmuls during the DMA prologue keep HAM at 2.4GHz
A_FP16 = 880          # per-expert cap of fp16 tokens (highest gate weight)
S_FP8 = 256.0         # host-side scale on fp8 weights (e4m3 normal range)

_program_cache: dict[tuple, object] = {}


def _token_chunks(C):
    """Equal-ish chunks (multiples of 4, <=512) covering C tokens."""
    nchunks = -(-C // 512)
    base = -(-C // nchunks)
    base = -(-base // 4) * 4
    chunks = []
    t = 0
    while t < C:
        n = min(base, C - t)
        chunks.append((t, n))
        t += n
    return chunks


def _build_program(C, C8):
    """Bass program for one expert's FFN: C fp16 tokens + C8 fp8 tokens."""
    key = (C, C8)
    if key in _program_cache:
        return _program_cache[key]

    fp16 = mybir.dt.float16
    fp32 = mybir.dt.float32
    fp8 = mybir.dt.float8e4
    DR = mybir.MatmulPerfMode.DoubleRow
    nc = bacc.Bacc("TRN2", target_bir_lowering=False, debug=False,
                   enable_asserts=True, num_devices=N_EXPERTS)

    chunks = _token_chunks(C)
    cmax = max(n for _, n in chunks)

    # DRAM inputs, pre-sliced host-side so every DMA is contiguous per row.
    # Chunk-0 tokens and w1's FIRST f-tile ride in ONE tensor ("crit").
    n0 = chunks[0][1]
    crit_d = nc.dram_tensor("crit", [P, KO, n0 + P], fp16,
                            kind="ExternalInput").ap()
    w1r_d = nc.dram_tensor("w1r", [P, KO, (FO_PER_W1 - 1) * P], fp16,
                           kind="ExternalInput").ap()
    xt_d = [None] + [
        nc.dram_tensor(f"xt{ci}", [P, KO, n], fp16, kind="ExternalInput").ap()
        for ci, (_, n) in list(enumerate(chunks))[1:]]
    w1_d = [None] + [
        nc.dram_tensor(f"w1_{s}", [P, KO, FO_PER_W1 * P], fp16,
                       kind="ExternalInput").ap()
        for s in range(1, FO // FO_PER_W1)]
    w2_d = [nc.dram_tensor(f"w2_{s}", [P, FO // W_PARTS, D_MODEL], fp16,
                           kind="ExternalInput").ap() for s in range(W_PARTS)]
    b1_d = nc.dram_tensor("b1c", [P, FO], fp32, kind="ExternalInput").ap()
    b2_d = nc.dram_tensor("b2c", [P, KO], fp32, kind="ExternalInput").ap()
    yt_d = nc.dram_tensor("yt", [P, KO, C], fp32, kind="ExternalOutput").ap()
    # fp8 tier: tokens/weights in DoubleRow layout (contraction pairs on a
    # middle 2-dim; effective K=256 per matmul instruction)
    xq_d = nc.dram_tensor("xq", [P, KO // 2, 2, C8], fp8,
                          kind="ExternalInput").ap()
    w1q_d = nc.dram_tensor("w1q", [P, KO // 2, 2, D_FF], fp8,
                           kind="ExternalInput").ap()
    w2q_d = nc.dram_tensor("w2q", [P, FO // 2, 2, D_MODEL], fp8,
                           kind="ExternalInput").ap()
    yt8_d = nc.dram_tensor("yt8", [P, KO, C8], fp16,
                           kind="ExternalOutput").ap()

    FO_PER_PART = FO // W_PARTS

    with tile.TileContext(nc) as tc:
        with (
            tc.tile_pool(name="wpool", bufs=1) as wpool,
            tc.tile_pool(name="hpool", bufs=2) as hpool,
            tc.tile_pool(name="ypool", bufs=2) as ypool,
            tc.tile_pool(name="pspool", bufs=4, space="PSUM") as pspool,
        ):
            crit_sb = wpool.tile([P, KO, n0 + P], fp16)
            w1r_sb = wpool.tile([P, KO, (FO_PER_W1 - 1) * P], fp16)
            xt_sb = [crit_sb[:, :, :n0]] + [
                wpool.tile([P, KO, n], fp16, name=f"xt_sb{ci}")
                for ci, (_, n) in list(enumerate(chunks))[1:]]
            w1_sb = [
                wpool.tile([P, KO, FO_PER_W1 * P], fp16, name=f"w1_sb{s}")
                for s in range(1, FO // FO_PER_W1)]

            def w1_tile(fo):
                """(tile, local f index) holding w1 f-tile `fo`."""
                if fo == 0:
                    return crit_sb[:, :, n0:], 0
                if fo < FO_PER_W1:
                    return w1r_sb, fo - 1
                return w1_sb[fo // FO_PER_W1 - 1], fo % FO_PER_W1
            w2_sb = [wpool.tile([P, FO_PER_PART, D_MODEL], fp16,
                                name=f"w2_sb{s}")
                     for s in range(W_PARTS)]
            b1_sb = wpool.tile([P, FO], fp32)
            b2_sb = wpool.tile([P, KO], fp32)
            xq_sb = wpool.tile([P, KO // 2, 2, C8], fp8)
            w1q_sb = wpool.tile([P, KO // 2, 2, D_FF], fp8)
            w2q_sb = wpool.tile([P, FO // 2, 2, D_MODEL], fp8)
            ht8_sb = wpool.tile([P, FO // 2, 2, C8], fp8)
            yt8_sb = wpool.tile([P, KO, C8], fp16)

            # PE warmup: dummy matmuls on a zeroed tile fill the DMA
            # prologue so the HAM clock-gate reaches 2.4GHz before the
            # real matmuls start.
            warm = wpool.tile([P, 512], fp16)
            nc.gpsimd.memset(warm[:], 0.0)
            ps_w = pspool.tile([P, 512], fp32, name="ps_w", bufs=1)
            for _ in range(WARMUP_MMS):
                nc.tensor.matmul(ps_w[:], lhsT=warm[:, :P], rhs=warm[:],
                                 start=True, stop=True)

            # DMA order = need order: the crit bundle (chunk-0 tokens +
            # first w1 f-tile) unblocks the first matmuls with a single
            # issue; the rest of w1, then w2, then the fp8 tier stream in.
            nc.sync.dma_start(crit_sb[:], crit_d[:])
            nc.sync.dma_start(w1r_sb[:], w1r_d[:])
            for s in range(1, FO // FO_PER_W1):
                nc.sync.dma_start(w1_sb[s - 1][:], w1_d[s][:])
                if s == 1:
                    nc.sync.dma_start(b1_sb[:], b1_d[:])
            for ci in range(1, len(chunks)):
                nc.sync.dma_start(xt_sb[ci][:], xt_d[ci][:])
            for s in range(W_PARTS):
                nc.sync.dma_start(w2_sb[s][:], w2_d[s][:])
            nc.sync.dma_start(b2_sb[:], b2_d[:])
            nc.sync.dma_start(xq_sb[:], xq_d[:])
            nc.sync.dma_start(w1q_sb[:], w1q_d[:])
            nc.sync.dma_start(w2q_sb[:], w2q_d[:])

            def emit_mm1_fp8():
                """fp8 MM1 (DoubleRow): 24 f-groups of 3 K=256 steps."""
                for fo in range(FO):
                    ps = pspool.tile([P, cmax], fp32, name="ps")
                    for k3 in range(KO // 2):
                        nc.tensor.matmul(
                            ps[:, :C8],
                            lhsT=w1q_sb[:, k3, :, fo * P:(fo + 1) * P],
                            rhs=xq_sb[:, k3, :, :C8],
                            start=(k3 == 0), stop=(k3 == KO // 2 - 1),
                            perf_mode=DR,
                        )
                    # descale the S_FP8 weight scale inside the relu epilogue
                    nc.scalar.activation(
                        ht8_sb[:, fo // 2, fo % 2, :C8], ps[:, :C8],
                        mybir.ActivationFunctionType.Relu,
                        bias=b1_sb[:, fo:fo + 1], scale=1.0 / S_FP8,
                    )

            def emit_mm2_fp8():
                """fp8 MM2 (DoubleRow): 6 ko-groups of 12 K=256 steps."""
                for ko in range(KO):
                    ps = pspool.tile([P, cmax], fp32, name="ps")
                    for f2 in range(FO // 2):
                        nc.tensor.matmul(
                            ps[:, :C8],
                            lhsT=w2q_sb[:, f2, :, ko * P:(ko + 1) * P],
                            rhs=ht8_sb[:, f2, :, :C8],
                            start=(f2 == 0), stop=(f2 == FO // 2 - 1),
                            perf_mode=DR,
                        )
                    if ko % 2:
                        nc.vector.tensor_copy(yt8_sb[:, ko, :C8],
                                              ps[:, :C8])
                        # stream the tail output on otherwise-idle queues
                        eng = (nc.sync, nc.gpsimd, nc.scalar)[ko // 2]
                        eng.dma_start(yt8_d[:, ko - 1:ko + 1, :],
                                      yt8_sb[:, ko - 1:ko + 1, :])
                    else:
                        nc.scalar.activation(
                            yt8_sb[:, ko, :C8], ps[:, :C8],
                            mybir.ActivationFunctionType.Copy)

            for ci, (t0, nt) in enumerate(chunks):
                lastc = ci == len(chunks) - 1
                ht = hpool.tile([P, FO, cmax], fp16, name="ht")
                for fo in range(FO):
                    w1t, f = w1_tile(fo)
                    ps = pspool.tile([P, cmax], fp32, name="ps")
                    for ko in range(KO):
                        nc.tensor.matmul(
                            ps[:, :nt],
                            lhsT=w1t[:, ko, f * P:(f + 1) * P],
                            rhs=xt_sb[ci][:, ko, :nt],
                            start=(ko == 0), stop=(ko == KO - 1),
                        )
                    nc.scalar.activation(
                        ht[:, fo, :nt], ps[:, :nt],
                        mybir.ActivationFunctionType.Relu,
                        bias=b1_sb[:, fo:fo + 1],
                    )
                if lastc:
                    # the fp8 MM1 fills the relu latency before the last
                    # fp16 chunk's MM2, and its MM2 forms the (cheap) tail
                    emit_mm1_fp8()
                yt = ypool.tile([P, KO, cmax], fp32, name="yt")
                for ko in range(KO):
                    ps = pspool.tile([P, cmax], fp32, name="ps")
                    for fo in range(FO):
                        s, f = divmod(fo, FO_PER_PART)
                        nc.tensor.matmul(
                            ps[:, :nt],
                            lhsT=w2_sb[s][:, f, ko * P:(ko + 1) * P],
                            rhs=ht[:, fo, :nt],
                            start=(fo == 0), stop=(fo == FO - 1),
                        )
                    # DVE is ~3x faster than ACT for the plain bias-add
                    # drain; the final one is on the critical tail.
                    nc.vector.tensor_scalar_add(
                        yt[:, ko, :nt], ps[:, :nt], b2_sb[:, ko:ko + 1])
                    nc.sync.dma_start(yt_d[:, ko, t0:t0 + nt], yt[:, ko, :nt])
            emit_mm2_fp8()

    nc.compile()
    _program_cache[key] = nc
    return nc


def _route(xf, Wr):
    """Host router: top-2 expert ids + softmax weights (matches lax.top_k)."""
    T = xf.shape[0]
    logits = xf @ Wr
    i1 = np.argmax(logits, axis=1)
    l1 = logits[np.arange(T), i1]
    masked = logits.copy()
    masked[np.arange(T), i1] = -np.inf
    i2 = np.argmax(masked, axis=1)
    l2 = logits[np.arange(T), i2]
    e2 = np.exp((l2 - l1).astype(np.float32))
    wt1 = 1.0 / (1.0 + e2)
    wt2 = e2 / (1.0 + e2)
    return i1, i2, wt1, wt2


def _forward(inputs, trace=False, trace_kwargs=None):
    x = np.ascontiguousarray(np.asarray(inputs["x"], dtype=np.float32))
    Wr = np.asarray(inputs["Wr"], dtype=np.float32)
    W1 = np.asarray(inputs["W1"], dtype=np.float32)
    b1 = np.asarray(inputs["b1"], dtype=np.float32)
    W2 = np.asarray(inputs["W2"], dtype=np.float32)
    b2 = np.asarray(inputs["b2"], dtype=np.float32)

    B, S, D = x.shape
    T = B * S
    xf = x.reshape(T, D)

    i1, i2, wt1, wt2 = _route(xf, Wr)
    fp8e4 = ml_dtypes.float8_e4m3

    idx16, idx8, gw16, gw8 = [], [], [], []
    for e in range(N_EXPERTS):
        ix = np.nonzero((i1 == e) | (i2 == e))[0]
        g = np.where(i1[ix] == e, wt1[ix], wt2[ix]).astype(np.float32)
        order = np.argsort(-g, kind="stable")
        s16, s8 = order[:A_FP16], order[A_FP16:]
        idx16.append(ix[s16]); gw16.append(g[s16])
        idx8.append(ix[s8]); gw8.append(g[s8])

    C = max(max(len(ix) for ix in idx16), 4)
    C = -(-C // 4) * 4
    C8 = max(max(len(ix) for ix in idx8), 4)
    C8 = -(-C8 // 4) * 4
    nc = _build_program(C, C8)
    chunks = _token_chunks(C)

    in_maps = []
    for e in range(N_EXPERTS):
        ix = idx16[e]
        xe = np.zeros((C, D), dtype=np.float16)
        xe[:len(ix)] = xf[ix]
        # XT[d,t] -> [p, ko, t] with d = ko*P + p
        xt = np.ascontiguousarray(xe.T.reshape(KO, P, C).transpose(1, 0, 2))
        w1 = np.ascontiguousarray(
            W1[e].astype(np.float16).reshape(KO, P, D_FF).transpose(1, 0, 2))
        w2 = np.ascontiguousarray(
            W2[e].astype(np.float16).reshape(FO, P, D_MODEL).transpose(1, 0, 2))
        m = {"b1c": np.ascontiguousarray(b1[e].reshape(FO, P).T),
             "b2c": np.ascontiguousarray(b2[e].reshape(KO, P).T)}
        n0 = chunks[0][1]
        m["crit"] = np.ascontiguousarray(
            np.concatenate([xt[:, :, :n0], w1[:, :, :P]], axis=2))
        m["w1r"] = np.ascontiguousarray(w1[:, :, P:FO_PER_W1 * P])
        for ci, (t0, n) in list(enumerate(chunks))[1:]:
            m[f"xt{ci}"] = np.ascontiguousarray(xt[:, :, t0:t0 + n])
        for s in range(1, FO // FO_PER_W1):
            f0 = s * FO_PER_W1 * P
            m[f"w1_{s}"] = np.ascontiguousarray(w1[:, :, f0:f0 + FO_PER_W1 * P])
        FO_PER_PART = FO // W_PARTS
        for s in range(W_PARTS):
            m[f"w2_{s}"] = np.ascontiguousarray(
                w2[:, s * FO_PER_PART:(s + 1) * FO_PER_PART, :])
        # fp8 tier: DoubleRow layouts, weights pre-scaled by S_FP8.
        # d = k3*256 + i*128 + p (MM1); f = f2*256 + i*128 + p (MM2)
        ix8 = idx8[e]
        xq = np.zeros((C8, D), dtype=np.float32)
        xq[:len(ix8)] = xf[ix8]
        m["xq"] = np.ascontiguousarray(
            xq.T.astype(fp8e4).reshape(KO // 2, 2, P, C8)
            .transpose(2, 0, 1, 3))
        m["w1q"] = np.ascontiguousarray(
            (W1[e] * S_FP8).astype(fp8e4).reshape(KO // 2, 2, P, D_FF)
            .transpose(2, 0, 1, 3))
        m["w2q"] = np.ascontiguousarray(
            (W2[e] * S_FP8).astype(fp8e4).reshape(FO // 2, 2, P, D_MODEL)
            .transpose(2, 0, 1, 3))
        in_maps.append(m)

    try:
        res = bass_utils.run_bass_kernel_spmd(
            nc, in_maps, core_ids=list(range(N_EXPERTS)), trace=trace,
            **(trace_kwargs or {}),
        )
    except Exception:
        # transient device errors (NRT_EXEC_UNIT_UNRECOVERABLE) have been
        # observed once under rapid successive loads; one retry clears them
        res = bass_utils.run_bass_kernel_spmd(
            nc, in_maps, core_ids=list(range(N_EXPERTS)), trace=trace,
            **(trace_kwargs or {}),
        )

    out = np.zeros((T, D), dtype=np.float32)
    for e in range(N_EXPERTS):
        ix = idx16[e]
        if len(ix):
            # yt [p, ko, t] -> Y [t, d]; b2 already added on device
            yt = res.results[e]["yt"]
            ye = yt.transpose(2, 1, 0).reshape(C, D)[:len(ix)]
            out[ix] += gw16[e][:, None] * ye
        ix8 = idx8[e]
        if len(ix8):
            yt8 = res.results[e]["yt8"].astype(np.float32)
            ye = yt8.transpose(2, 1, 0).reshape(C8, D)[:len(ix8)] / S_FP8
            out[ix8] += gw8[e][:, None] * (ye + b2[e][None, :])
    return out.reshape(B, S, D), res


def kernel(**inputs) -> np.ndarray:
    out, _ = _forward(inputs)
    return out
